# revision 24
# baseline (speedup 1.0000x reference)
"""Trainium2 Bass kernel for nn_L2LossDif (pairwise L2 contrastive loss).

Math (see the algebraic reduction in the problem's reference):
    sq_m  = sum(feats_m ** 2)           (scalar, per matrix)
    mu_m  = feats_m.sum(axis=0)         ([D], per matrix)
then a handful of scalar ops combine sq_n, sq_a, mu_n, mu_a into the loss.

Strategy: data-parallel row shard across 8 cores (1024 rows of each matrix
per core). Each core streams its 16 MiB of rows once from HBM on the SP
HWDGE queue (1 MiB chunks of 128 rows). Per chunk:
  - sum of squares : ScalarE Square activation, accum_out -> rsq column
  - cols    0:1536 : TensorE ones-matmul (f32r) x3 -> [1,1536] PSUM
                     accumulator per matrix (3 PSUM banks each)
  - cols 1536:2048 : VectorE adds into acc_hi [128,512] (copy on chunk 0),
                     partition-reduced by one ones-matmul at end of matrix
                     into a [1,512] PSUM tile (1 bank each; 8 banks total)
With deep chunk buffering (bufs=10) the DMA stream runs at ~440 GB/s and
every engine keeps pace.  PSUM->SBUF copies chase each group's stop
matmul in [1,512] pieces so the tail overlaps.  Outputs are tiny
(mu [1,4096] + rsq [128,16]) and leave on the SP queue at the end.
Partition/core reductions and the scalar combine run on the host in f64.
"""

import numpy as np

import concourse.bacc as bacc
import concourse.mybir as mybir
import concourse.tile as tile
from concourse.bass_utils import run_bass_kernel_spmd

N_CORES = 8
N_ROWS_FULL = 8192
D = 2048
P = 128
ROWS = N_ROWS_FULL // N_CORES  # rows per core per matrix
NCHUNK = ROWS // P  # chunks per matrix (128-row tiles)
MM_N = 512  # moving free dim per matmul (one PSUM bank)
D_PE = 3 * MM_N  # columns summed on TensorE; the rest accumulate on VectorE

_NC_CACHE = {}


def build_module():
    nc = bacc.Bacc("TRN2", target_bir_lowering=False, debug=False)
    f32 = mybir.dt.float32
    f32r = mybir.dt.float32r
    bf16 = mybir.dt.bfloat16
    srcs = [
        nc.dram_tensor("nfeats", [ROWS, D], f32, kind="ExternalInput"),
        nc.dram_tensor("afeats", [ROWS, D], f32, kind="ExternalInput"),
    ]
    out_mu = nc.dram_tensor("mu", [1, 2 * D], f32, kind="ExternalOutput")
    out_rsq = nc.dram_tensor("rsq", [P, 2 * NCHUNK], f32, kind="ExternalOutput")

    with tile.TileContext(nc) as tc:
        with (
            tc.tile_pool(name="chunks", bufs=10) as chunk_pool,
            tc.tile_pool(name="sq", bufs=2) as sq_pool,
            tc.tile_pool(name="psum", bufs=1, space="PSUM") as psum_pool,
            tc.tile_pool(name="small", bufs=1) as small_pool,
        ):
            rsq_all = small_pool.tile([P, 2 * NCHUNK], f32)
            mu_sb = small_pool.tile([1, 2 * D], f32)
            ones = small_pool.tile([P, 1], f32)
            nc.gpsimd.memset(ones, 1.0)
            ones_r = ones.bitcast(f32r)

            for m, src in enumerate(srcs):
                psum_mu = psum_pool.tile([1, D_PE], f32, tag=f"mu{m}")
                psum_hi = psum_pool.tile([1, MM_N], f32, tag=f"hi{m}")
                acc_hi = small_pool.tile([P, MM_N], f32r, tag=f"acchi{m}")
                last = m == len(srcs) - 1

                for c in range(NCHUNK):
                    chunk = chunk_pool.tile([P, D], f32r)
                    # Alternate input chunks between the two HWDGE queues
                    # (SP / Activation): two descriptor streams give this
                    # core a larger outstanding-request footprint in the
                    # device DMA arbiter when all 8 cores contend for HBM.
                    eng = nc.sync if (m * NCHUNK + c) % 2 == 0 else nc.scalar
                    eng.dma_start(
                        out=chunk,
                        in_=src[c * P : (c + 1) * P, :].bitcast(f32r),
                    )
                    sq = sq_pool.tile([P, D], bf16)
                    nc.scalar.activation(
                        out=sq,
                        in_=chunk.bitcast(f32),
                        func=mybir.ActivationFunctionType.Square,
                        accum_out=rsq_all[:, m * NCHUNK + c : m * NCHUNK + c + 1],
                    )
                    hi = chunk[:, D_PE:D]
                    if c == 0:
                        nc.vector.tensor_copy(acc_hi, hi)
                    else:
                        nc.vector.tensor_add(acc_hi, acc_hi, hi)
                    for j in range(D_PE // MM_N):
                        nc.tensor.matmul(
                            psum_mu[0:1, j * MM_N : (j + 1) * MM_N],
                            lhsT=ones_r,
                            rhs=chunk[:, j * MM_N : (j + 1) * MM_N],
                            start=(c == 0),
                            stop=(c == NCHUNK - 1),
                        )
                        if last and c == NCHUNK - 1:
                            # Chase each group's stop matmul with its copy so
                            # the tail overlaps the final matmuls.
                            nc.vector.tensor_copy(
                                mu_sb[0:1, m * D + j * MM_N : m * D + (j + 1) * MM_N],
                                psum_mu[0:1, j * MM_N : (j + 1) * MM_N],
                            )
                nc.tensor.matmul(
                    psum_hi, lhsT=ones_r, rhs=acc_hi, start=True, stop=True
                )
                if not last:
                    nc.vector.tensor_copy(mu_sb[0:1, m * D : m * D + D_PE], psum_mu)
                nc.vector.tensor_copy(
                    mu_sb[0:1, m * D + D_PE : (m + 1) * D], psum_hi
                )

            # Tiny output DMAs at the end of the SP queue (idle once the
            # input stream has been dispatched).  rsq is ready before the
            # last mu copy, so it goes first.
            nc.sync.dma_start(out=out_rsq[:, :], in_=rsq_all)
            nc.sync.dma_start(out=out_mu[:, :], in_=mu_sb)
    nc.compile()
    return nc


def get_module():
    if "nc" not in _NC_CACHE:
        _NC_CACHE["nc"] = build_module()
    return _NC_CACHE["nc"]


def kernel(nfeats, afeats):
    nfeats = np.asarray(nfeats, dtype=np.float32)
    afeats = np.asarray(afeats, dtype=np.float32)
    assert nfeats.shape == (N_ROWS_FULL, D) and afeats.shape == (N_ROWS_FULL, D)

    nc = get_module()
    in_maps = [
        {
            "nfeats": np.ascontiguousarray(nfeats[c * ROWS : (c + 1) * ROWS]),
            "afeats": np.ascontiguousarray(afeats[c * ROWS : (c + 1) * ROWS]),
        }
        for c in range(N_CORES)
    ]
    results = run_bass_kernel_spmd(nc, in_maps, core_ids=list(range(N_CORES))).results

    mu = np.zeros((2, D), dtype=np.float64)
    sq = np.zeros(2, dtype=np.float64)
    for r in results:
        mu += np.asarray(r["mu"], dtype=np.float64).reshape(2, D)
        rsq = np.asarray(r["rsq"], dtype=np.float64)
        sq[0] += rsq[:, :NCHUNK].sum()
        sq[1] += rsq[:, NCHUNK:].sum()

    return combine(mu[0], mu[1], sq[0], sq[1])


def combine(mu_n, mu_a, sq_n, sq_a):
    nnum = anum = float(N_ROWS_FULL)
    nsum = nnum * sq_n - float(mu_n @ mu_n)
    asum = anum * sq_a - float(mu_a @ mu_a)
    cross_sum = anum * sq_n + nnum * sq_a - 2.0 * float(mu_n @ mu_a)

    ncount = nnum * (nnum - 1) / 2
    acount = anum * (anum - 1) / 2
    count = nnum * anum

    loss_dif = cross_sum / count
    within = (asum + nsum) / (acount + ncount)
    loss = -np.log(loss_dif / (loss_dif + within))
    return np.asarray(loss, dtype=np.float32)
